# revision 40
# baseline (speedup 1.0000x reference)
"""Performer (FAVOR+) multi-head fast-attention TRN2 kernel — self-contained.

Problem: B=4, N=4096, D=1024, H=16, M=256, DH=64.
Sharding: 2 heads per core (head-parallel attention) on 8 NeuronCores;
on-device AllToAll re-shards to sequence-parallel for the output Linear
(row-parallel, no partial sums); host stitches the 8 n-shards.

All matmul traffic is bf16 (PE runs 1 col/cycle vs 4 for fp32-HIGH);
accumulation stays fp32 in PSUM.  Stabilizers that cancel in the
num/den ratio are dropped; the k-side row max and ||k||^2 factors are
folded into v, matching the reference up to float rounding.

Pipeline: the kernel runs as 8 "slots" (one per batch x head-pair).
The scalar engine (exp) is the binding resource, so each slot emits
its own k-feature exps first and the PREVIOUS slot's q-feature exps
second, keeping scalar 100% busy.  ctx/out matmuls of the previous
slot fill the PE pacing gaps; the per-slot AllToAll and the output
Linear of earlier batches are interleaved behind.
"""
import contextlib
import sys

sys.path.insert(0, "/opt/trn_rl_repo")

import numpy as np
import ml_dtypes

import concourse.bacc as bacc
import concourse.mybir as mybir
from concourse.tile import TileContext
from concourse.bass_utils import run_bass_kernel_spmd
F32 = mybir.dt.float32
BF16 = mybir.dt.bfloat16
AF = mybir.ActivationFunctionType
ALU = mybir.AluOpType
NPBF16 = ml_dtypes.bfloat16

NCORES = 8
B, N, D = 4, 4096, 1024
H, M, DH = 16, 256, 64
T = N // 128          # 32 token tiles of 128
J = N // 512          # 8 query blocks of 512
NS = N // NCORES      # 512 tokens per core after resharding
DS = float(DH) ** -0.25

_CACHE = {}


def _build():
    nc = bacc.Bacc(num_devices=NCORES)
    groups = [list(range(NCORES))]

    qkT = nc.declare_dram_parameter("qkT", [B, 2, 128, N], BF16, isOutput=False)
    knvn = nc.declare_dram_parameter("knvn", [B, 128, T, 256], BF16, isOutput=False)
    projKZ = nc.declare_dram_parameter("projKZ", [128, M], BF16, isOutput=False)
    projQZ = nc.declare_dram_parameter("projQZ", [128, M], BF16, isOutput=False)
    WT = nc.declare_dram_parameter("WT", [128, NCORES, D], BF16, isOutput=False)
    ident = nc.declare_dram_parameter("ident", [128, 128], F32, isOutput=False)
    out_ext = nc.declare_dram_parameter("out", [B, NS, D], F32, isOutput=True)

    h_in = nc.dram_tensor("h_in", [B, NCORES, 2, 65, NS], BF16)
    h_out = nc.dram_tensor("h_out", [B, NCORES, 2, 65, NS], BF16)
    dinv_scr = nc.dram_tensor("dinv_scr", [B, 16, NS], BF16)
    warm_in = nc.dram_tensor("warm_in", [NCORES, 64], BF16)
    warm_out = nc.dram_tensor("warm_out", [NCORES, 64], BF16)

    with TileContext(nc) as tc:
        with contextlib.ExitStack() as stk:
            const_p = stk.enter_context(tc.tile_pool(name="const", bufs=1))
            qkT_p = stk.enter_context(tc.tile_pool(name="qkT", bufs=3))
            knvn_p = stk.enter_context(tc.tile_pool(name="knvn", bufs=2))
            ek_p = stk.enter_context(tc.tile_pool(name="ek", bufs=2))
            small_p = stk.enter_context(tc.tile_pool(name="small", bufs=3))
            vaug_p = stk.enter_context(tc.tile_pool(name="vaug", bufs=2))
            qpt_p = stk.enter_context(tc.tile_pool(name="qpt", bufs=4))
            stg_p = stk.enter_context(tc.tile_pool(name="stg", bufs=2))
            hx_p = stk.enter_context(tc.tile_pool(name="hx", bufs=1))
            hgn_p = stk.enter_context(tc.tile_pool(name="hgn", bufs=2))
            oc_p = stk.enter_context(tc.tile_pool(name="oc", bufs=2))
            # PSUM budget (8 banks): mm 2x2 (k- and q-feature tiles,
            # double buffered) + ctx 1 + po 1 + pl 2x1; transposes
            # borrow mm tiles.
            ps_mm = stk.enter_context(tc.tile_pool(name="psmm", bufs=2, space="PSUM"))
            ps_ctx = stk.enter_context(tc.tile_pool(name="psctx", bufs=1, space="PSUM"))
            ps_po = stk.enter_context(tc.tile_pool(name="pspo", bufs=1, space="PSUM"))
            ps_pl = stk.enter_context(tc.tile_pool(name="pspl", bufs=2, space="PSUM"))

            # -- warmup collective: primes the CC rings / absorbs core skew
            warm_sb = const_p.tile([NCORES, 64], BF16, tag="warm")
            nc.gpsimd.memset(warm_sb[:], 0.0)
            nc.sync.dma_start(out=warm_in[:], in_=warm_sb[:])
            nc.gpsimd.collective_compute(
                "AllToAll", ALU.bypass, replica_groups=groups,
                ins=[warm_in[:]], outs=[warm_out[:]])



            state = {}

            def emit_knvn_load(b):
                knvn_sb = knvn_p.tile([128, T, 256], BF16, tag="knvn")
                for pp in range(4):
                    nc.sync.dma_start(out=knvn_sb[32 * pp:32 * (pp + 1), :, :],
                                      in_=knvn[b, 32 * pp:32 * (pp + 1), :, :])
                state[("knvn", b)] = knvn_sb

            def emit_ksq_dn(b):
                knvn_sb = state[("knvn", b)]
                ksl = knvn_sb[:, :, 0:128]
                nc.gpsimd.tensor_tensor(out=ksl, in0=ksl, in1=ksl, op=ALU.mult)

            def emit_qkT_load(b, h):
                qkT_sb = qkT_p.tile([128, N], BF16, tag="qkT")
                for pp in range(2):
                    nc.sync.dma_start(out=qkT_sb[64 * pp:64 * (pp + 1), :],
                                      in_=qkT[b, h, 64 * pp:64 * (pp + 1), :])
                state[("qkT", b, h)] = qkT_sb

            def emit_post_dma(b, dma_eng=None):
                # After the AllToAll of batch b: fetch numerators + dens,
                # build 1/den broadcast, scale -> hgn ready for the Linear.
                eng = dma_eng if dma_eng is not None else nc.sync
                hraw = hx_p.tile([128, NCORES, NS], BF16, tag="hraw")
                for hh in range(2):
                    eng.dma_start(
                        out=hraw[DH * hh:DH * (hh + 1), :, :],
                        in_=h_out[b, :, hh, 0:DH, :].rearrange("c d n -> d c n"))
                den16 = small_p.tile([16, NS], BF16, tag="den16")
                eng.dma_start(
                    out=den16[:],
                    in_=h_out[b, :, :, DH, :].rearrange("c h n -> (c h) n"))
                denf = small_p.tile([16, NS], F32, tag="denf")
                nc.vector.tensor_copy(denf[:], den16[:])
                dinv16 = small_p.tile([16, NS], BF16, tag="dinv16")
                with nc.allow_low_precision("bf16 1/den is plenty here"):
                    nc.vector.reciprocal(dinv16[:], denf[:])
                eng.dma_start(out=dinv_scr[b], in_=dinv16[:])
                dinvB = hx_p.tile([128, NCORES, NS], BF16, tag="dinvB")
                for hh in range(2):
                    eng.dma_start(
                        out=dinvB[DH * hh:DH * (hh + 1), :, :],
                        in_=dinv_scr[b].rearrange("(c h) n -> c h n", h=2)[:, hh, :]
                            .unsqueeze(0).broadcast_to([DH, NCORES, NS]))
                hgn = hgn_p.tile([128, NCORES, NS], BF16, tag="hgn")
                nc.vector.tensor_tensor(out=hgn[:], in0=hraw[:], in1=dinvB[:],
                                        op=ALU.mult)
                state[("hgn", b)] = hgn

            def emit_lin_group(b, g):
                # one PSUM accumulation group of the output Linear of batch b
                hgn = state[("hgn", b)]
                nci, oh = g // 2, g % 2
                if oh == 0:
                    oc_new = oc_p.tile([128, 2, 512], F32, tag="oc", name="oc")
                    state[("oc", b, nci)] = oc_new
                oc = state[("oc", b, nci)]
                pl = ps_pl.tile([128, 512], F32, tag="pl")
                for cc in range(NCORES):
                    nc.tensor.matmul(
                        pl[:], hgn[:, cc, 128 * nci:128 * (nci + 1)],
                        WT_sb[:, cc, 512 * oh:512 * (oh + 1)],
                        start=(cc == 0), stop=(cc == NCORES - 1),
                        skip_group_check=True)
                nc.vector.tensor_copy(oc[:, oh, :], pl[:])
                if oh == 1:
                    nc.sync.dma_start(
                        out=out_ext[b, 128 * nci:128 * (nci + 1), :],
                        in_=oc[:].rearrange("p a f -> p (a f)"))

            # ---- per-slot pieces -------------------------------------
            def emit_kf_step(s, tb):
                # 4 k-feature MMs into a feat tile + exp + me chain
                b, h = s
                qkT_sb = state[("qkT", b, h)]
                ek = state[("ek", s)]
                me = state[("me", s)]
                pf = ps_mm.tile([128, 2, 512], F32, tag="mm", name="pf")
                pf4 = pf[:].rearrange("p a (c f) -> p (a c) f", c=2)
                for qq in range(4):
                    t = 4 * tb + qq
                    nc.tensor.matmul(
                        pf4[:, qq, :], qkT_sb[:, 128 * t:128 * (t + 1)],
                        projKZ_sb[:],
                        start=True, stop=True, skip_group_check=True)
                nc.scalar.activation(
                    ek[:, 4 * tb:4 * (tb + 1), :], pf4[:], AF.Exp, scale=DS)
                if tb <= 4:
                    emit_me_chunk(s, tb)

            def emit_me_chunk(s, tb):
                ek = state[("ek", s)]
                me = state[("me", s)]
                nc.vector.tensor_reduce(
                    out=me[:, 4 * tb:4 * (tb + 1)],
                    in_=ek[:, 4 * tb:4 * (tb + 1), :],
                    axis=mybir.AxisListType.X, op=ALU.max)

            def emit_gq(s):
                # g = exp(-dn') / me, then vaug = [v * g | g]
                b, h = s
                knvn_sb = state[("knvn", b)]
                me = state[("me", s)]
                eg = state[("eg", s)]
                rme = state[("rme", s)]
                g_t = state[("g", s)]
                vaug = state[("vaug", s)]
                nc.vector.reciprocal(rme[:], me[:])
                nc.vector.tensor_tensor(
                    out=g_t[:], in0=eg[:], in1=rme[:], op=ALU.mult)
                nc.gpsimd.tensor_tensor(
                    out=vaug[:, :, 0:DH],
                    in0=knvn_sb[:, :, 128 + DH * h:128 + DH * (h + 1)],
                    in1=g_t[:].rearrange("p (t one) -> p t one", one=1)
                        .broadcast_to([128, T, DH]),
                    op=ALU.mult)
                nc.gpsimd.tensor_copy(vaug[:, :, DH], g_t[:])

            def emit_ctx_group(s, gi):
                # 4 ctx accumulation MMs (tiles 4*gi..4*gi+3) of slot s
                ek = state[("ek", s)]
                vaug = state[("vaug", s)]
                pctx = state[("pctx", s)]
                for t in range(4 * gi, 4 * gi + 4):
                    nc.tensor.matmul(
                        pctx[:], vaug[:, t, :], ek[:, t, :],
                        start=(t == 0), stop=(t == T - 1), skip_group_check=True)

            def emit_trans(s):
                pctx = state.pop(("pctx", s))
                ctxs = small_p.tile([65, 256], F32, tag="ctxs")
                nc.vector.tensor_copy(ctxs[:], pctx[:])
                ctxT = small_p.tile([128, 2, 65], BF16, tag="ctxT")
                pf_t = ps_mm.tile([128, 2, 512], F32, tag="mm", name="pf_t")
                for mi in range(2):
                    ptv = pf_t[:, mi, 0:65]
                    nc.tensor.transpose(ptv, ctxs[:, 128 * mi:128 * (mi + 1)],
                                        ident_sb[0:65, 0:65])
                    nc.vector.tensor_copy(ctxT[:, mi, :], ptv)
                state[("ctxT", s)] = ctxT

            def emit_qf(s, j):
                # q-feature MMs + exp for block j of slot s
                b, h = s
                qkT_sb = state[("qkT", b, h)]
                pq = ps_mm.tile([128, 2, 512], F32, tag="mm", name="pq")
                for mi in range(2):
                    nc.tensor.matmul(
                        pq[:, mi, :],
                        projQZ_sb[:, 128 * mi:128 * (mi + 1)],
                        qkT_sb[:, 512 * j:512 * (j + 1)],
                        start=True, stop=True, skip_group_check=True)
                qpt = qpt_p.tile([128, 2, 512], BF16, tag="qpt")
                nc.scalar.activation(qpt[:], pq[:], AF.Exp, scale=DS)
                state[("qpt", s, j)] = qpt

            def emit_out(s, j):
                b, h = s
                ctxT = state[("ctxT", s)]
                qpt = state.pop(("qpt", s, j))
                stg = state[("stg", s)]
                po = ps_po.tile([65, 512], F32, tag="po", name="po")
                for mi in range(2):
                    nc.tensor.matmul(
                        po[:], ctxT[:, mi, :], qpt[:, mi, :],
                        start=(mi == 0), stop=(mi == 1), skip_group_check=True)
                if j % 2 == 0:
                    nc.vector.tensor_copy(stg[:, j, :], po[:])
                else:
                    nc.scalar.activation(stg[:, j, :], po[:], AF.Copy)

            def emit_slot_open(s):
                b, h = s
                ek = ek_p.tile([128, T, M], BF16, tag="ek", name="ek")
                state[("ek", s)] = ek
                state[("me", s)] = small_p.tile([128, T], F32, tag="me", name="me")
                state[("rme", s)] = small_p.tile([128, T], F32, tag="rme", name="rme")
                g_new = small_p.tile([128, T], BF16, tag="g", name="g")
                state[("g", s)] = g_new
                vaug = vaug_p.tile([128, T, 65], BF16, tag="vaug", name="vaug")
                state[("vaug", s)] = vaug
                knvn_sb = state[("knvn", b)]
                dn_h = small_p.tile([128, T], F32, tag="dn", name="dn_h")
                nc.vector.tensor_reduce(
                    out=dn_h[:],
                    in_=knvn_sb[:, :, DH * h:DH * (h + 1)],
                    axis=mybir.AxisListType.X, op=ALU.add)
                eg = small_p.tile([128, T], F32, tag="eg", name="eg")
                nc.scalar.activation(eg[:], dn_h[:], AF.Exp,
                                     scale=-0.5 * DS * DS)
                state[("eg", s)] = eg

            def emit_slot_close_prev(prev):
                # stg of prev is complete: ship it; after the second head
                # of a batch, trigger the batch's AllToAll
                b, h = prev
                stg = state[("stg", prev)]
                nc.sync.dma_start(
                    out=h_in[b, :, h].rearrange("c p n -> p c n"), in_=stg[:])
                if h == 1:
                    nc.gpsimd.collective_compute(
                        "AllToAll", ALU.bypass, replica_groups=groups,
                        ins=[h_in[b]], outs=[h_out[b]])

            # ---- slot schedule ---------------------------------------
            # slot sigma = 2b + h.  In slot sigma we emit:
            #   kF(sigma) [scalar: ek exps], ctx(sigma-1), trans(sigma-1),
            #   qF(sigma-1) [scalar: qpt exps], out(sigma-1), lin hooks.
            slots = [(b, h) for b in range(B) for h in range(2)]

            def lin_hooks_for(sigma):
                # linear groups of batch bb: 4 at slot 2bb+4, 4 at 2bb+5
                for bb in range(B):
                    if sigma == 2 * bb + 4:
                        return [(bb, g) for g in range(4)]
                    if sigma == 2 * bb + 5:
                        return [(bb, g) for g in range(4, 8)]
                return []

            # -- head: first slot's inputs first, heavyweight consts later
            projKZ_sb = const_p.tile([128, M], BF16, tag="projKZ")
            nc.sync.dma_start(out=projKZ_sb[:], in_=projKZ[:])
            projQZ_sb = const_p.tile([128, M], BF16, tag="projQZ")
            nc.sync.dma_start(out=projQZ_sb[:], in_=projQZ[:])
            emit_qkT_load(0, 0)
            emit_knvn_load(0)
            ident_sb = const_p.tile([128, 128], F32, tag="ident")
            nc.sync.dma_start(out=ident_sb[:], in_=ident[:])
            WT_sb = const_p.tile([128, NCORES, D], BF16, tag="WT")
            nc.sync.dma_start(out=WT_sb[:], in_=WT[:])
            emit_ksq_dn(0)

            for sigma, s in enumerate(slots):
                b, h = s
                prev = slots[sigma - 1] if sigma > 0 else None
                if h == 0:
                    if b + 1 < B:
                        emit_knvn_load(b + 1)
                elif b + 1 < B:
                    emit_ksq_dn(b + 1)
                emit_slot_open(s)
                # prefetch next slot's qk tile
                if sigma + 1 < len(slots):
                    emit_qkT_load(*slots[sigma + 1])
                if prev is not None:
                    state[("stg", prev)] = stg_p.tile(
                        [65, J, 512], BF16, tag="stg", name="stg")
                    state[("pctx", prev)] = ps_ctx.tile(
                        [65, 256], F32, tag="ctx", name="pctx")
                lins = lin_hooks_for(sigma)

                # interleave: kF steps + prev ctx groups + first prev qF
                for tb in range(8):
                    emit_kf_step(s, tb)
                    if prev is not None:
                        if tb >= 2:
                            emit_ctx_group(prev, tb - 2)
                        if tb == 5:
                            emit_qf(prev, 0)
                        if tb == 6:
                            emit_qf(prev, 1)
                if prev is not None:
                    emit_ctx_group(prev, 6)
                    emit_ctx_group(prev, 7)
                    emit_trans(prev)
                    for j in range(J):
                        if j >= 2:
                            emit_qf(prev, j)
                        emit_out(prev, j)
                        if j < 3:
                            emit_me_chunk(s, 5 + j)
                        if j == 2:
                            emit_gq(s)
                        if j == 0 and b > 0 and h == 1:
                            # A2A(b-1) done by now -> build hgn(b-1)
                            emit_post_dma(b - 1)
                        if lins and j < len(lins):
                            emit_lin_group(*lins[j])
                    emit_slot_close_prev(prev)
                else:
                    for tb in range(5, 8):
                        emit_me_chunk(s, tb)
                    emit_gq(s)

            # ---- tail: last slot's ctx/q/out + final linear ----------
            last = slots[-1]
            state[("stg", last)] = stg_p.tile([65, J, 512], BF16, tag="stg",
                                              name="stg")
            state[("pctx", last)] = ps_ctx.tile([65, 256], F32, tag="ctx",
                                                name="pctx")
            for gi in range(8):
                emit_ctx_group(last, gi)
            emit_trans(last)
            for j in range(J):
                emit_qf(last, j)
                emit_out(last, j)
            emit_slot_close_prev(last)
            # lin(2) first: its PE work and vector copies must not sit
            # behind post(3)'s vector ops, which wait on the last AllToAll
            for g in range(8):
                emit_lin_group(2, g)
            emit_post_dma(B - 1, dma_eng=nc.gpsimd)
            for g in range(8):
                emit_lin_group(B - 1, g)

    nc.compile()
    return nc


def _get_nc():
    if "nc" not in _CACHE:
        _CACHE["nc"] = _build()
    return _CACHE["nc"]


def _host_prep(q, k, v, W):
    qb = q.astype(NPBF16)
    kb = k.astype(NPBF16)
    vb = v.astype(NPBF16)
    # W.T rearranged: WT[p, cc, o] = W[o, cc*128 + p]
    WTh = np.ascontiguousarray(
        W.T.astype(NPBF16).reshape(NCORES, 128, D).transpose(1, 0, 2))
    identity = np.eye(128, dtype=np.float32)
    in_maps = []
    for c in range(NCORES):
        lo = c * 128
        qc = qb[:, :, lo:lo + 128]   # [B, N, 128]
        kc = kb[:, :, lo:lo + 128]
        vc = vb[:, :, lo:lo + 128]
        # [B, 2, 64, N] transposed per head-pair
        kT = kc.reshape(B, N, 2, DH).transpose(0, 2, 3, 1)
        qT = qc.reshape(B, N, 2, DH).transpose(0, 2, 3, 1)
        qkTh = np.ascontiguousarray(
            np.concatenate([kT, qT], axis=2))   # [B, 2, 128, N]
        kn = kc.reshape(B, T, 128, 128).transpose(0, 2, 1, 3)
        vn = vc.reshape(B, T, 128, 128).transpose(0, 2, 1, 3)
        knvnh = np.ascontiguousarray(
            np.concatenate([kn, vn], axis=3))   # [B, 128, T, 256]
        in_maps.append({
            "qkT": qkTh,
            "knvn": knvnh,
            "projKZ": None,   # filled below (shared)
            "projQZ": None,
            "WT": WTh,
            "ident": identity,
        })
    return in_maps


def kernel(q, k, v, W, b, proj, _profile=False):
    q = np.asarray(q, np.float32)
    k = np.asarray(k, np.float32)
    v = np.asarray(v, np.float32)
    W = np.asarray(W, np.float32)
    b = np.asarray(b, np.float32)
    proj = np.asarray(proj, np.float32)

    nc = _get_nc()
    in_maps = _host_prep(q, k, v, W)
    projT = np.ascontiguousarray(proj.T.astype(NPBF16))      # [64, M]
    zer = np.zeros_like(projT)
    projKZ = np.concatenate([projT, zer], axis=0)            # [128, M]
    projQZ = np.concatenate([zer, projT], axis=0)
    for m in in_maps:
        m["projKZ"] = projKZ
        m["projQZ"] = projQZ
    res = run_bass_kernel_spmd(nc, in_maps, list(range(NCORES)), trace=_profile)
    out = np.empty((B, N, D), dtype=np.float32)
    for c in range(NCORES):
        out[:, c * NS:(c + 1) * NS, :] = res.results[c]["out"]
    out += b
    if _profile:
        _CACHE["last_exec_time_ns"] = res.exec_time_ns
        _CACHE["last_profile_json"] = res.profile_json
    return out


# revision 42
# speedup vs baseline: 1.1186x; 1.1186x over previous
"""Performer (FAVOR+) multi-head fast-attention TRN2 kernel — self-contained.

Problem: B=4, N=4096, D=1024, H=16, M=256, DH=64.
Sharding: 2 heads per core (head-parallel attention) on 8 NeuronCores;
on-device AllToAll re-shards to sequence-parallel for the output Linear
(row-parallel, no partial sums); host stitches the 8 n-shards.

All matmul traffic is bf16 (PE runs 1 col/cycle vs 4 for fp32-HIGH);
accumulation stays fp32 in PSUM.  Stabilizers that cancel in the
num/den ratio are dropped; the k-side row max and ||k||^2 factors are
folded into v, matching the reference up to float rounding.

Pipeline: the kernel runs as 8 "slots" (one per batch x head-pair).
The scalar engine (exp) is the binding resource, so each slot emits
its own k-feature exps first and the PREVIOUS slot's q-feature exps
second, keeping scalar 100% busy.  ctx/out matmuls of the previous
slot fill the PE pacing gaps; the per-slot AllToAll and the output
Linear of earlier batches are interleaved behind.
"""
import contextlib
import sys

sys.path.insert(0, "/opt/trn_rl_repo")

import numpy as np
import ml_dtypes

import concourse.bacc as bacc
import concourse.mybir as mybir
from concourse.tile import TileContext
from concourse.bass_utils import run_bass_kernel_spmd
F32 = mybir.dt.float32
BF16 = mybir.dt.bfloat16
AF = mybir.ActivationFunctionType
ALU = mybir.AluOpType
NPBF16 = ml_dtypes.bfloat16

NCORES = 8
B, N, D = 4, 4096, 1024
H, M, DH = 16, 256, 64
T = N // 128          # 32 token tiles of 128
J = N // 512          # 8 query blocks of 512
NS = N // NCORES      # 512 tokens per core after resharding
DS = float(DH) ** -0.25

_CACHE = {}


def _build():
    nc = bacc.Bacc(num_devices=NCORES)
    groups = [list(range(NCORES))]

    qkT = nc.declare_dram_parameter("qkT", [B, 2, 128, N], BF16, isOutput=False)
    knvn = nc.declare_dram_parameter("knvn", [B, 128, T, 256], BF16, isOutput=False)
    projKZ = nc.declare_dram_parameter("projKZ", [128, M], BF16, isOutput=False)
    projQZ = nc.declare_dram_parameter("projQZ", [128, M], BF16, isOutput=False)
    WT = nc.declare_dram_parameter("WT", [128, NCORES, D], BF16, isOutput=False)
    ident = nc.declare_dram_parameter("ident", [128, 128], F32, isOutput=False)
    out_ext = nc.declare_dram_parameter("out", [B, NS, D], F32, isOutput=True)

    h_in = nc.dram_tensor("h_in", [B, NCORES, 2, 65, NS], BF16)
    h_out = nc.dram_tensor("h_out", [B, NCORES, 2, 65, NS], BF16)
    dinv_scr = nc.dram_tensor("dinv_scr", [B, 16, NS], BF16)
    warm_in = nc.dram_tensor("warm_in", [NCORES, 64], BF16)
    warm_out = nc.dram_tensor("warm_out", [NCORES, 64], BF16)

    with TileContext(nc) as tc:
        with contextlib.ExitStack() as stk:
            const_p = stk.enter_context(tc.tile_pool(name="const", bufs=1))
            qkT_p = stk.enter_context(tc.tile_pool(name="qkT", bufs=3))
            knvn_p = stk.enter_context(tc.tile_pool(name="knvn", bufs=2))
            ek_p = stk.enter_context(tc.tile_pool(name="ek", bufs=2))
            small_p = stk.enter_context(tc.tile_pool(name="small", bufs=3))
            vaug_p = stk.enter_context(tc.tile_pool(name="vaug", bufs=2))
            qpt_p = stk.enter_context(tc.tile_pool(name="qpt", bufs=4))
            stg_p = stk.enter_context(tc.tile_pool(name="stg", bufs=2))
            hx_p = stk.enter_context(tc.tile_pool(name="hx", bufs=1))
            hgn_p = stk.enter_context(tc.tile_pool(name="hgn", bufs=2))
            oc_p = stk.enter_context(tc.tile_pool(name="oc", bufs=2))
            # PSUM budget (8 banks): mm 2x2 (k- and q-feature tiles,
            # double buffered) + ctx 1 + po 1 + pl 2x1; transposes
            # borrow mm tiles.
            ps_mm = stk.enter_context(tc.tile_pool(name="psmm", bufs=2, space="PSUM"))
            ps_ctx = stk.enter_context(tc.tile_pool(name="psctx", bufs=1, space="PSUM"))
            ps_po = stk.enter_context(tc.tile_pool(name="pspo", bufs=1, space="PSUM"))
            ps_pl = stk.enter_context(tc.tile_pool(name="pspl", bufs=2, space="PSUM"))

            # -- warmup collective: primes the CC rings / absorbs core skew
            warm_sb = const_p.tile([NCORES, 64], BF16, tag="warm")
            nc.gpsimd.memset(warm_sb[:], 0.0)
            nc.sync.dma_start(out=warm_in[:], in_=warm_sb[:])
            nc.gpsimd.collective_compute(
                "AllToAll", ALU.bypass, replica_groups=groups,
                ins=[warm_in[:]], outs=[warm_out[:]])



            state = {}

            def emit_knvn_load(b):
                knvn_sb = knvn_p.tile([128, T, 256], BF16, tag="knvn")
                for pp in range(4):
                    nc.sync.dma_start(out=knvn_sb[32 * pp:32 * (pp + 1), :, :],
                                      in_=knvn[b, 32 * pp:32 * (pp + 1), :, :])
                state[("knvn", b)] = knvn_sb

            def emit_ksq_dn(b):
                knvn_sb = state[("knvn", b)]
                ksl = knvn_sb[:, :, 0:128]
                nc.gpsimd.tensor_tensor(out=ksl, in0=ksl, in1=ksl, op=ALU.mult)

            def emit_qkT_load(b, h):
                qkT_sb = qkT_p.tile([128, N], BF16, tag="qkT")
                for pp in range(2):
                    nc.sync.dma_start(out=qkT_sb[64 * pp:64 * (pp + 1), :],
                                      in_=qkT[b, h, 64 * pp:64 * (pp + 1), :])
                state[("qkT", b, h)] = qkT_sb

            def emit_post_dma(b, dma_eng=None):
                # After the AllToAll of batch b: fetch numerators + dens,
                # build 1/den broadcast, scale -> hgn ready for the Linear.
                eng = dma_eng if dma_eng is not None else nc.sync
                hraw = hx_p.tile([128, NCORES, NS], BF16, tag="hraw")
                for hh in range(2):
                    eng.dma_start(
                        out=hraw[DH * hh:DH * (hh + 1), :, :],
                        in_=h_out[b, :, hh, 0:DH, :].rearrange("c d n -> d c n"))
                den16 = small_p.tile([16, NS], BF16, tag="den16")
                eng.dma_start(
                    out=den16[:],
                    in_=h_out[b, :, :, DH, :].rearrange("c h n -> (c h) n"))
                denf = small_p.tile([16, NS], F32, tag="denf")
                nc.vector.tensor_copy(denf[:], den16[:])
                dinv16 = small_p.tile([16, NS], BF16, tag="dinv16")
                with nc.allow_low_precision("bf16 1/den is plenty here"):
                    nc.vector.reciprocal(dinv16[:], denf[:])
                eng.dma_start(out=dinv_scr[b], in_=dinv16[:])
                dinvB = hx_p.tile([128, NCORES, NS], BF16, tag="dinvB")
                for hh in range(2):
                    eng.dma_start(
                        out=dinvB[DH * hh:DH * (hh + 1), :, :],
                        in_=dinv_scr[b].rearrange("(c h) n -> c h n", h=2)[:, hh, :]
                            .unsqueeze(0).broadcast_to([DH, NCORES, NS]))
                hgn = hgn_p.tile([128, NCORES, NS], BF16, tag="hgn")
                nc.vector.tensor_tensor(out=hgn[:], in0=hraw[:], in1=dinvB[:],
                                        op=ALU.mult)
                state[("hgn", b)] = hgn

            def emit_lin_group(b, g):
                # one PSUM accumulation group of the output Linear of batch b
                hgn = state[("hgn", b)]
                nci, oh = g // 2, g % 2
                if oh == 0:
                    oc_new = oc_p.tile([128, 2, 512], F32, tag="oc", name="oc")
                    state[("oc", b, nci)] = oc_new
                oc = state[("oc", b, nci)]
                pl = ps_pl.tile([128, 512], F32, tag="pl")
                for cc in range(NCORES):
                    nc.tensor.matmul(
                        pl[:], hgn[:, cc, 128 * nci:128 * (nci + 1)],
                        WT_sb[:, cc, 512 * oh:512 * (oh + 1)],
                        start=(cc == 0), stop=(cc == NCORES - 1),
                        skip_group_check=True)
                nc.vector.tensor_copy(oc[:, oh, :], pl[:])
                if oh == 1:
                    nc.sync.dma_start(
                        out=out_ext[b, 128 * nci:128 * (nci + 1), :],
                        in_=oc[:].rearrange("p a f -> p (a f)"))

            # ---- per-slot pieces -------------------------------------
            def emit_kf_step(s, tb):
                # 4 k-feature MMs into a feat tile + exp + me chain
                b, h = s
                qkT_sb = state[("qkT", b, h)]
                ek = state[("ek", s)]
                me = state[("me", s)]
                pf = ps_mm.tile([128, 2, 512], F32, tag="mm", name="pf")
                pf4 = pf[:].rearrange("p a (c f) -> p (a c) f", c=2)
                for qq in range(4):
                    t = 4 * tb + qq
                    nc.tensor.matmul(
                        pf4[:, qq, :], qkT_sb[:, 128 * t:128 * (t + 1)],
                        projKZ_sb[:],
                        start=True, stop=True, skip_group_check=True)
                nc.scalar.activation(
                    ek[:, 4 * tb:4 * (tb + 1), :], pf4[:], AF.Exp, scale=DS)
                if tb <= 4:
                    emit_me_chunk(s, tb)

            def emit_me_chunk(s, tb):
                ek = state[("ek", s)]
                me = state[("me", s)]
                nc.vector.tensor_reduce(
                    out=me[:, 4 * tb:4 * (tb + 1)],
                    in_=ek[:, 4 * tb:4 * (tb + 1), :],
                    axis=mybir.AxisListType.X, op=ALU.max)

            def emit_gq(s):
                # g = exp(-dn') / me, then vaug = [v * g | g]
                b, h = s
                knvn_sb = state[("knvn", b)]
                me = state[("me", s)]
                eg = state[("eg", s)]
                rme = state[("rme", s)]
                g_t = state[("g", s)]
                vaug = state[("vaug", s)]
                nc.vector.reciprocal(rme[:], me[:])
                nc.vector.tensor_tensor(
                    out=g_t[:], in0=eg[:], in1=rme[:], op=ALU.mult)
                nc.gpsimd.tensor_tensor(
                    out=vaug[:, :, 0:DH],
                    in0=knvn_sb[:, :, 128 + DH * h:128 + DH * (h + 1)],
                    in1=g_t[:].rearrange("p (t one) -> p t one", one=1)
                        .broadcast_to([128, T, DH]),
                    op=ALU.mult)
                nc.gpsimd.tensor_copy(vaug[:, :, DH], g_t[:])

            def emit_ctx_group(s, gi):
                # 4 ctx accumulation MMs (tiles 4*gi..4*gi+3) of slot s
                ek = state[("ek", s)]
                vaug = state[("vaug", s)]
                pctx = state[("pctx", s)]
                for t in range(4 * gi, 4 * gi + 4):
                    nc.tensor.matmul(
                        pctx[:], vaug[:, t, :], ek[:, t, :],
                        start=(t == 0), stop=(t == T - 1), skip_group_check=True)

            def emit_trans(s):
                pctx = state.pop(("pctx", s))
                ctxs = small_p.tile([65, 256], F32, tag="ctxs")
                nc.vector.tensor_copy(ctxs[:], pctx[:])
                ctxT = small_p.tile([128, 2, 65], BF16, tag="ctxT")
                pf_t = ps_mm.tile([128, 2, 512], F32, tag="mm", name="pf_t")
                for mi in range(2):
                    ptv = pf_t[:, mi, 0:65]
                    nc.tensor.transpose(ptv, ctxs[:, 128 * mi:128 * (mi + 1)],
                                        ident_sb[0:65, 0:65])
                    nc.vector.tensor_copy(ctxT[:, mi, :], ptv)
                state[("ctxT", s)] = ctxT

            def emit_qf(s, j):
                # q-feature MMs + exp for block j of slot s
                b, h = s
                qkT_sb = state[("qkT", b, h)]
                pq = ps_mm.tile([128, 2, 512], F32, tag="mm", name="pq")
                for mi in range(2):
                    nc.tensor.matmul(
                        pq[:, mi, :],
                        projQZ_sb[:, 128 * mi:128 * (mi + 1)],
                        qkT_sb[:, 512 * j:512 * (j + 1)],
                        start=True, stop=True, skip_group_check=True)
                qpt = qpt_p.tile([128, 2, 512], BF16, tag="qpt")
                nc.scalar.activation(qpt[:], pq[:], AF.Exp, scale=DS)
                state[("qpt", s, j)] = qpt

            def emit_out(s, j):
                b, h = s
                ctxT = state[("ctxT", s)]
                qpt = state.pop(("qpt", s, j))
                stg = state[("stg", s)]
                po = ps_po.tile([65, 512], F32, tag="po", name="po")
                for mi in range(2):
                    nc.tensor.matmul(
                        po[:], ctxT[:, mi, :], qpt[:, mi, :],
                        start=(mi == 0), stop=(mi == 1), skip_group_check=True)
                if j % 2 == 0:
                    nc.vector.tensor_copy(stg[:, j, :], po[:])
                else:
                    nc.scalar.activation(stg[:, j, :], po[:], AF.Copy)

            def emit_slot_open(s):
                b, h = s
                ek = ek_p.tile([128, T, M], BF16, tag="ek", name="ek")
                state[("ek", s)] = ek
                state[("me", s)] = small_p.tile([128, T], F32, tag="me", name="me")
                state[("rme", s)] = small_p.tile([128, T], F32, tag="rme", name="rme")
                g_new = small_p.tile([128, T], BF16, tag="g", name="g")
                state[("g", s)] = g_new
                vaug = vaug_p.tile([128, T, 65], BF16, tag="vaug", name="vaug")
                state[("vaug", s)] = vaug
                knvn_sb = state[("knvn", b)]
                dn_h = small_p.tile([128, T], F32, tag="dn", name="dn_h")
                nc.vector.tensor_reduce(
                    out=dn_h[:],
                    in_=knvn_sb[:, :, DH * h:DH * (h + 1)],
                    axis=mybir.AxisListType.X, op=ALU.add)
                eg = small_p.tile([128, T], F32, tag="eg", name="eg")
                nc.scalar.activation(eg[:], dn_h[:], AF.Exp,
                                     scale=-0.5 * DS * DS)
                state[("eg", s)] = eg

            def emit_slot_close_prev(prev):
                # stg of prev is complete: ship it; after the second head
                # of a batch, trigger the batch's AllToAll
                b, h = prev
                stg = state[("stg", prev)]
                nc.sync.dma_start(
                    out=h_in[b, :, h].rearrange("c p n -> p c n"), in_=stg[:])
                if h == 1:
                    nc.gpsimd.collective_compute(
                        "AllToAll", ALU.bypass, replica_groups=groups,
                        ins=[h_in[b]], outs=[h_out[b]])

            # ---- slot schedule ---------------------------------------
            # slot sigma = 2b + h.  In slot sigma we emit:
            #   kF(sigma) [scalar: ek exps], ctx(sigma-1), trans(sigma-1),
            #   qF(sigma-1) [scalar: qpt exps], out(sigma-1), lin hooks.
            slots = [(b, h) for b in range(B) for h in range(2)]

            def lin_hooks_for(sigma):
                # linear groups of batch bb: 4 at slot 2bb+4, 4 at 2bb+5
                for bb in range(B):
                    if sigma == 2 * bb + 4:
                        return [(bb, g) for g in range(4)]
                    if sigma == 2 * bb + 5:
                        return [(bb, g) for g in range(4, 8)]
                return []

            # -- head: first slot's inputs first, heavyweight consts later
            projKZ_sb = const_p.tile([128, M], BF16, tag="projKZ")
            nc.sync.dma_start(out=projKZ_sb[:], in_=projKZ[:])
            projQZ_sb = const_p.tile([128, M], BF16, tag="projQZ")
            nc.sync.dma_start(out=projQZ_sb[:], in_=projQZ[:])
            emit_qkT_load(0, 0)
            emit_knvn_load(0)
            ident_sb = const_p.tile([128, 128], F32, tag="ident")
            nc.sync.dma_start(out=ident_sb[:], in_=ident[:])
            WT_sb = const_p.tile([128, NCORES, D], BF16, tag="WT")
            nc.sync.dma_start(out=WT_sb[:], in_=WT[:])
            emit_ksq_dn(0)

            for sigma, s in enumerate(slots):
                b, h = s
                prev = slots[sigma - 1] if sigma > 0 else None
                if h == 0:
                    if b + 1 < B:
                        emit_knvn_load(b + 1)
                elif b + 1 < B:
                    emit_ksq_dn(b + 1)
                emit_slot_open(s)
                # prefetch next slot's qk tile
                if sigma + 1 < len(slots):
                    emit_qkT_load(*slots[sigma + 1])
                if prev is not None:
                    state[("stg", prev)] = stg_p.tile(
                        [65, J, 512], BF16, tag="stg", name="stg")
                    state[("pctx", prev)] = ps_ctx.tile(
                        [65, 256], F32, tag="ctx", name="pctx")
                lins = lin_hooks_for(sigma)

                # interleave: kF steps + prev ctx groups + first prev qF
                for tb in range(8):
                    emit_kf_step(s, tb)
                    if prev is not None:
                        if tb >= 2:
                            emit_ctx_group(prev, tb - 2)
                        if tb == 5:
                            emit_qf(prev, 0)
                        if tb == 6:
                            emit_qf(prev, 1)
                if prev is not None:
                    emit_ctx_group(prev, 6)
                    emit_ctx_group(prev, 7)
                    emit_trans(prev)
                    for j in range(J):
                        if j >= 2:
                            emit_qf(prev, j)
                        emit_out(prev, j)
                        if j < 3:
                            emit_me_chunk(s, 5 + j)
                        if j == 2:
                            emit_gq(s)
                        if j == 0 and b > 0 and h == 1:
                            # A2A(b-1) done by now -> build hgn(b-1)
                            emit_post_dma(b - 1)
                        if lins and j < len(lins):
                            emit_lin_group(*lins[j])
                    emit_slot_close_prev(prev)
                else:
                    for tb in range(5, 8):
                        emit_me_chunk(s, tb)
                    emit_gq(s)

            # ---- tail: last slot's ctx/q/out + final linear ----------
            last = slots[-1]
            state[("stg", last)] = stg_p.tile([65, J, 512], BF16, tag="stg",
                                              name="stg")
            state[("pctx", last)] = ps_ctx.tile([65, 256], F32, tag="ctx",
                                                name="pctx")
            for gi in range(8):
                emit_ctx_group(last, gi)
            emit_trans(last)
            for j in range(J):
                emit_qf(last, j)
                emit_out(last, j)
            emit_slot_close_prev(last)
            # lin(2) first: its PE work and vector copies must not sit
            # behind post(3)'s vector ops, which wait on the last AllToAll
            for g in range(8):
                emit_lin_group(2, g)
            emit_post_dma(B - 1, dma_eng=nc.gpsimd)
            for g in range(8):
                emit_lin_group(B - 1, g)

    nc.compile()
    return nc


def _get_nc():
    if "nc" not in _CACHE:
        _CACHE["nc"] = _build()
    return _CACHE["nc"]


def _host_prep(q, k, v, W):
    qb = q.astype(NPBF16)
    kb = k.astype(NPBF16)
    vb = v.astype(NPBF16)
    # W.T rearranged: WT[p, cc, o] = W[o, cc*128 + p]
    WTh = np.ascontiguousarray(
        W.T.astype(NPBF16).reshape(NCORES, 128, D).transpose(1, 0, 2))
    identity = np.eye(128, dtype=np.float32)
    in_maps = []
    for c in range(NCORES):
        lo = c * 128
        qc = qb[:, :, lo:lo + 128]   # [B, N, 128]
        kc = kb[:, :, lo:lo + 128]
        vc = vb[:, :, lo:lo + 128]
        # [B, 2, 64, N] transposed per head-pair
        kT = kc.reshape(B, N, 2, DH).transpose(0, 2, 3, 1)
        qT = qc.reshape(B, N, 2, DH).transpose(0, 2, 3, 1)
        qkTh = np.ascontiguousarray(
            np.concatenate([kT, qT], axis=2))   # [B, 2, 128, N]
        kn = kc.reshape(B, T, 128, 128).transpose(0, 2, 1, 3)
        vn = vc.reshape(B, T, 128, 128).transpose(0, 2, 1, 3)
        knvnh = np.ascontiguousarray(
            np.concatenate([kn, vn], axis=3))   # [B, 128, T, 256]
        in_maps.append({
            "qkT": qkTh,
            "knvn": knvnh,
            "projKZ": None,   # filled below (shared)
            "projQZ": None,
            "WT": WTh,
            "ident": identity,
        })
    return in_maps


def kernel(q, k, v, W, b, proj, _profile=False):
    q = np.asarray(q, np.float32)
    k = np.asarray(k, np.float32)
    v = np.asarray(v, np.float32)
    W = np.asarray(W, np.float32)
    b = np.asarray(b, np.float32)
    proj = np.asarray(proj, np.float32)

    nc = _get_nc()
    in_maps = _host_prep(q, k, v, W)
    projT = np.ascontiguousarray(proj.T.astype(NPBF16))      # [64, M]
    zer = np.zeros_like(projT)
    projKZ = np.concatenate([projT, zer], axis=0)            # [128, M]
    projQZ = np.concatenate([zer, projT], axis=0)
    for m in in_maps:
        m["projKZ"] = projKZ
        m["projQZ"] = projQZ
    res = run_bass_kernel_spmd(nc, in_maps, list(range(NCORES)), trace=_profile)
    out = np.empty((B, N, D), dtype=np.float32)
    for c in range(NCORES):
        out[:, c * NS:(c + 1) * NS, :] = res.results[c]["out"]
    out += b
    if _profile:
        _CACHE["last_exec_time_ns"] = res.exec_time_ns
        _CACHE["last_profile_json"] = res.profile_json
    return out
